# revision 11
# baseline (speedup 1.0000x reference)
"""AttnLSTMEncoder Trainium2 kernel.

Computes, for src (B=64, T=512) int tokens:
    x  = emb[src]                       (B, T, E)
    xg = x @ Wx.T + bx                  (B, T, 4H)
    LSTM over T: gates = xg_t + h @ Wh.T ; i,f,o,g split; c = f*c + i*g;
                 h = o * tanh(c)
Returns (outputs (B,T,H) f32, h_T (B,H) f32, c_T (B,H) f32).

Strategy: data-parallel over batch across 8 NeuronCores (8 sequences per
core).  All on-chip tensors are kept transposed (H/4H dim on the 128 SBUF
partitions, batch on the free dim) so the scalar/vector engines run at full
lane utilization.  The recurrent matmul uses Wh tiles as the stationary
operand in bf16 (fast weight load), accumulating each gate into its own PSUM
bank so activations overlap the tensor engine.  The input-gate precompute
xg is done per 64-step block into SBUF (no DRAM round trip): embedding rows
are gathered transposed via dma_gather directly into (128, E/128, 512) bf16.
"""

import sys

for _p in ("/opt/trn_rl_repo", "/opt/trn_rl_repo/concourse"):
    if _p not in sys.path:
        sys.path.insert(0, _p)

import numpy as np
import ml_dtypes

import concourse.bass as bass
import concourse.bacc as bacc
import concourse.mybir as mybir
import concourse.tile as tile
from concourse.bass import ds

AF = mybir.ActivationFunctionType
ALU = mybir.AluOpType
F32 = mybir.dt.float32
BF16 = mybir.dt.bfloat16
I16 = mybir.dt.int16

P = 128
B = 64          # total batch
NCORES = 8
BL = B // NCORES  # batch per core = 8
T = 512
E = 1024
H = 1024
G4 = 4 * H      # 4096
KC = E // P     # 8 contraction chunks
MC = G4 // P    # 32 output chunks of the gate dim
VOCAB = 32000

# Physical gate order on the 4H axis (chosen so the PE computes the gates in
# the order the elementwise chain consumes them: g, i, f, o).
# reference order in Wx/Wh rows: i, f, o, g  ->  physical: g, i, f, o
PHYS_GATES = (3, 0, 1, 2)  # orig block index for each physical block


def _build_program(n_blocks: int, tb: int, unroll: int):
    """Emit the bass/Tile program. tb = timesteps per block."""
    tok = tb * BL  # tokens (= matmul free dim) per block

    nc = bacc.Bacc(trn_type="TRN2", target_bir_lowering=False)

    emb_d = nc.dram_tensor("emb", [VOCAB, E], BF16, kind="ExternalInput")
    whT_d = nc.dram_tensor("whT", [KC, P, G4], BF16, kind="ExternalInput")
    wxT_d = nc.dram_tensor("wxT", [KC, P, G4], BF16, kind="ExternalInput")
    bxT_d = nc.dram_tensor("bxT", [P, MC], F32, kind="ExternalInput")
    idx_d = nc.dram_tensor("idx", [n_blocks, P, tok // 16], I16,
                           kind="ExternalInput")
    out_hT_d = nc.dram_tensor("out_hT", [n_blocks * tb * P, BL * KC], F32,
                              kind="ExternalOutput")
    out_c_d = nc.dram_tensor("out_c", [P, KC * BL], F32, kind="ExternalOutput")

    with tile.TileContext(nc) as tc:
        with (
            tc.tile_pool(name="const", bufs=1) as cpool,
            tc.tile_pool(name="state", bufs=1) as spool,
            tc.tile_pool(name="xt", bufs=2) as xtpool,
            tc.tile_pool(name="idx", bufs=2) as idxpool,
            tc.tile_pool(name="work", bufs=3) as wpool,
            tc.tile_pool(name="psxg", bufs=2, space="PSUM") as psxg_pool,
            tc.tile_pool(name="psg", bufs=1, space="PSUM") as psg_pool,
        ):
            # ---- resident tensors ----
            whT = cpool.tile([P, KC, G4], BF16)
            nc.sync.dma_start(whT[:], whT_d[:].rearrange("k p f -> p k f"))
            wxT = cpool.tile([P, KC, G4], BF16)
            nc.sync.dma_start(wxT[:], wxT_d[:].rearrange("k p f -> p k f"))
            bxT = cpool.tile([P, MC], F32)
            nc.sync.dma_start(bxT[:], bxT_d[:])

            xgT = spool.tile([P, MC, tok], BF16)

            # ping-pong state: step k writes slot k%2, reads slot (k+1)%2
            c_s = [spool.tile([P, KC, BL], F32, tag=f"c{s}", name=f"c{s}")
                   for s in range(2)]
            hT_s = [spool.tile([P, KC, BL], BF16, tag=f"h{s}", name=f"h{s}")
                    for s in range(2)]
            nc.vector.memset(c_s[1][:], 0.0)
            nc.vector.memset(hT_s[1][:], 0.0)

            def step(t_expr, k_parity, stage_slot):
                """One LSTM timestep. t_expr: dynamic step index within block
                (RuntimeValue or int), k_parity: global step parity.
                stage_slot: (P, KC, BL) f32 AP to write h into."""
                h_prev = hT_s[(k_parity + 1) % 2]
                c_prev = c_s[(k_parity + 1) % 2]
                c_new = c_s[k_parity]
                h_new_bf = hT_s[k_parity]

                # 4 per-gate PSUM tiles (each its own bank)
                ps = {}
                for gi, gname in enumerate("gifo"):
                    pt = psg_pool.tile([P, KC, BL], F32, tag=f"ps_{gname}")
                    ps[gname] = pt
                    for hc in range(KC):
                        mc = gi * KC + hc
                        for kc in range(KC):
                            nc.tensor.matmul(
                                pt[:, hc, :],
                                lhsT=whT[:, kc, mc * P:(mc + 1) * P],
                                rhs=h_prev[:, kc, :],
                                start=(kc == 0),
                                stop=(kc == KC - 1),
                            )

                tok0 = t_expr * BL

                def pre(gname, gi):
                    pr = wpool.tile([P, KC, BL], F32, tag=f"pre_{gname}")
                    nc.vector.tensor_tensor(
                        pr[:], ps[gname][:],
                        xgT[:, gi * KC:(gi + 1) * KC, ds(tok0, BL)],
                        ALU.add)
                    return pr

                # g gate: tanh
                g_pre = pre("g", 0)
                g_t = wpool.tile([P, KC, BL], F32, tag="g_t")
                nc.scalar.activation(g_t[:], g_pre[:], AF.Tanh)
                # i gate: sigmoid; m1 = i*g
                i_pre = pre("i", 1)
                i_s = wpool.tile([P, KC, BL], F32, tag="i_s")
                nc.scalar.activation(i_s[:], i_pre[:], AF.Sigmoid)
                m1 = wpool.tile([P, KC, BL], F32, tag="m1")
                nc.vector.tensor_tensor(m1[:], i_s[:], g_t[:], ALU.mult)
                # f gate: sigmoid; m2 = f*c_prev; c_new = m1 + m2
                f_pre = pre("f", 2)
                f_s = wpool.tile([P, KC, BL], F32, tag="f_s")
                nc.scalar.activation(f_s[:], f_pre[:], AF.Sigmoid)
                m2 = wpool.tile([P, KC, BL], F32, tag="m2")
                nc.vector.tensor_tensor(m2[:], f_s[:], c_prev[:], ALU.mult)
                nc.vector.tensor_tensor(c_new[:], m1[:], m2[:], ALU.add)
                # tanh(c)
                tc_t = wpool.tile([P, KC, BL], F32, tag="tc_t")
                nc.scalar.activation(tc_t[:], c_new[:], AF.Tanh)
                # o gate: sigmoid; h = o * tanh(c)
                o_pre = pre("o", 3)
                o_s = wpool.tile([P, KC, BL], F32, tag="o_s")
                nc.scalar.activation(o_s[:], o_pre[:], AF.Sigmoid)
                nc.vector.tensor_tensor(stage_slot, o_s[:], tc_t[:], ALU.mult)
                # bf16 copy for next step's matmul rhs
                nc.vector.tensor_copy(h_new_bf[:], stage_slot)

            for kb in range(n_blocks):
                # ---- gather block embeddings, transposed, in bf16 ----
                idx_t = idxpool.tile([P, tok // 16], I16)
                nc.sync.dma_start(idx_t[:], idx_d[kb])
                xT = xtpool.tile([P, KC, tok], BF16)
                nc.gpsimd.dma_gather(
                    xT[:], emb_d[:], idx_t[:],
                    num_idxs=tok, num_idxs_reg=tok,
                    elem_size=E, transpose=True,
                )
                # ---- phase A: xgT = WxT.T @ xT + bx for the block ----
                for mc in range(MC):
                    pxg = psxg_pool.tile([P, tok], F32, tag="ps_xg")
                    for kc in range(KC):
                        nc.tensor.matmul(
                            pxg[:],
                            lhsT=wxT[:, kc, mc * P:(mc + 1) * P],
                            rhs=xT[:, kc, :],
                            start=(kc == 0),
                            stop=(kc == KC - 1),
                        )
                    nc.scalar.activation(
                        xgT[:, mc, :], pxg[:], AF.Identity,
                        bias=bxT[:, mc:mc + 1], scale=1.0)

                # ---- phase B: tb recurrent steps ----
                assert tb % unroll == 0 and unroll % 2 == 0

                def body(it_expr):
                    stage = wpool.tile([P, unroll, KC, BL], F32, tag="stage")
                    for u in range(unroll):
                        step(it_expr * unroll + u, u % 2, stage[:, u])
                    # one batched output DMA per body:
                    # dest rows (t, p) for t in [t0, t0+unroll)
                    row0 = (it_expr * unroll + kb * tb) * P
                    dst = out_hT_d[ds(row0, unroll * P), :]
                    dst = dst.rearrange("(u p) f -> p u f", p=P)
                    nc.sync.dma_start(
                        dst, stage[:].rearrange("p u a b -> p u (a b)"))

                if tb // unroll > 1:
                    with tc.For_i(0, tb // unroll, 1) as it:
                        body(it)
                else:
                    body(0)

            # final c state lives in slot (last step parity) = 1 for even tb*n
            total_steps = n_blocks * tb
            final = (total_steps - 1) % 2
            nc.sync.dma_start(
                out_c_d[:], c_s[final][:].rearrange("p a b -> p (a b)"))

    nc.compile()
    return nc


_PROG_CACHE = {}


def _get_program(n_blocks, tb, unroll):
    key = (n_blocks, tb, unroll)
    if key not in _PROG_CACHE:
        _PROG_CACHE[key] = _build_program(n_blocks, tb, unroll)
    return _PROG_CACHE[key]


def prep_core_inputs(src_c, emb_bf, whT_np, wxT_np, bxT_np, n_blocks, tb):
    """Per-core host-side input prep. src_c: (BL, n_blocks*tb) int."""
    tok = tb * BL
    # token order within block: (t_local, b)
    idx = np.empty((n_blocks, P, tok // 16), dtype=np.int16)
    for kb in range(n_blocks):
        flat = src_c[:, kb * tb:(kb + 1) * tb].T.reshape(-1)  # (tb, BL) -> tok
        wrapped = flat.reshape(tok // 16, 16).T.astype(np.int16)  # [p, s]
        # the 8 GPSIMD Q7 cores each read their own 16-partition group
        idx[kb] = np.tile(wrapped, (P // 16, 1))
    return {
        "emb": emb_bf,
        "whT": whT_np,
        "wxT": wxT_np,
        "bxT": bxT_np,
        "idx": idx,
    }


def prep_weights(emb, Wx, bx, Wh):
    """Host-side weight permutation / transposition / cast (shared by cores)."""
    emb_bf = np.ascontiguousarray(emb.astype(ml_dtypes.bfloat16))

    def permute_rows(w):
        blocks = [w[g * H:(g + 1) * H] for g in range(4)]
        return np.concatenate([blocks[g] for g in PHYS_GATES], axis=0)

    Wh_p = permute_rows(Wh)          # (4H, H)
    Wx_p = permute_rows(Wx)          # (4H, E)
    bx_p = permute_rows(bx.reshape(4, H)).reshape(-1) \
        if False else np.concatenate([bx[g * H:(g + 1) * H] for g in PHYS_GATES])

    # whT[kc, p, j] = Wh_p[j, kc*P + p]
    whT_np = np.ascontiguousarray(
        Wh_p.T.reshape(KC, P, G4).astype(ml_dtypes.bfloat16))
    wxT_np = np.ascontiguousarray(
        Wx_p.T.reshape(KC, P, G4).astype(ml_dtypes.bfloat16))
    # bxT[p, mc] = bx_p[mc*P + p]
    bxT_np = np.ascontiguousarray(bx_p.reshape(MC, P).T.astype(np.float32))
    return emb_bf, whT_np, wxT_np, bxT_np


def postprocess(out_hT, out_c, n_blocks, tb):
    """out_hT: (n_blocks*tb*P, BL*KC) f32 -> (BL, T, H); out_c -> (BL, H)."""
    t_total = n_blocks * tb
    a = out_hT.reshape(t_total, P, KC, BL)          # [t, p, hc, b]
    outputs = np.ascontiguousarray(
        a.transpose(3, 0, 2, 1).reshape(BL, t_total, H))
    c = out_c.reshape(P, KC, BL).transpose(2, 1, 0).reshape(BL, H)
    return outputs, np.ascontiguousarray(c)


def _install_trace_hook():
    """The image's antenv package lacks axon_hooks; recreate it so
    run_bass_kernel_spmd(trace=True) can capture NTFF profiles."""
    import types
    if "antenv.axon_hooks" in sys.modules:
        return
    mod = types.ModuleType("antenv.axon_hooks")
    _h = [None]
    mod.set_axon_ntff_profile_hook = lambda h: _h.__setitem__(0, h)
    mod.get_axon_ntff_profile_hook = lambda: _h[0]
    sys.modules["antenv.axon_hooks"] = mod
    try:
        import antenv
        antenv.axon_hooks = mod
    except ImportError:
        pass
    try:
        from trn_agent_boot.trn_boot import _ntff_profile_via_ctypes
        mod.set_axon_ntff_profile_hook(
            _ntff_profile_via_ctypes("/opt/axon/libaxon_pjrt.so"))
    except Exception:
        pass


def kernel(src, emb, Wx, bx, Wh, *, n_blocks=8, tb=64, unroll=4, trace=False):
    from concourse.bass_utils import run_bass_kernel_spmd

    if trace:
        _install_trace_hook()

    src = np.asarray(src)
    emb = np.asarray(emb, dtype=np.float32)
    Wx = np.asarray(Wx, dtype=np.float32)
    bx = np.asarray(bx, dtype=np.float32)
    Wh = np.asarray(Wh, dtype=np.float32)

    t_total = n_blocks * tb
    assert src.shape == (B, T) and t_total <= T

    emb_bf, whT_np, wxT_np, bxT_np = prep_weights(emb, Wx, bx, Wh)

    nc = _get_program(n_blocks, tb, unroll)
    in_maps = []
    for c in range(NCORES):
        src_c = src[c * BL:(c + 1) * BL, :t_total]
        in_maps.append(prep_core_inputs(
            src_c, emb_bf, whT_np, wxT_np, bxT_np, n_blocks, tb))

    res = run_bass_kernel_spmd(
        nc, in_maps, core_ids=list(range(NCORES)), trace=trace)

    outs = np.empty((B, t_total, H), dtype=np.float32)
    c_T = np.empty((B, H), dtype=np.float32)
    for c in range(NCORES):
        o_c, c_c = postprocess(
            res.results[c]["out_hT"], res.results[c]["out_c"], n_blocks, tb)
        outs[c * BL:(c + 1) * BL] = o_c
        c_T[c * BL:(c + 1) * BL] = c_c
    h_T = np.ascontiguousarray(outs[:, -1, :])
    if trace:
        kernel.last_exec_time_ns = res.exec_time_ns
    return outs, h_T, c_T


# revision 16
# speedup vs baseline: 1.0523x; 1.0523x over previous
"""AttnLSTMEncoder Trainium2 kernel.

Computes, for src (B=64, T=512) int tokens:
    x  = emb[src]                       (B, T, E)
    xg = x @ Wx.T + bx                  (B, T, 4H)
    LSTM over T: gates = xg_t + h @ Wh.T ; i,f,o,g split; c = f*c + i*g;
                 h = o * tanh(c)
Returns (outputs (B,T,H) f32, h_T (B,H) f32, c_T (B,H) f32).

Strategy: data-parallel over batch across 8 NeuronCores (8 sequences per
core).  All on-chip tensors are kept transposed (H/4H dim on the 128 SBUF
partitions, batch on the free dim) so the scalar/vector engines run at full
lane utilization.  The recurrent matmul uses Wh tiles as the stationary
operand in bf16 (fast weight load), accumulating each gate into its own PSUM
bank so activations overlap the tensor engine.  The input-gate precompute
xg is done per 64-step block into SBUF (no DRAM round trip): embedding rows
are gathered transposed via dma_gather directly into (128, E/128, 512) bf16.
"""

import sys

for _p in ("/opt/trn_rl_repo", "/opt/trn_rl_repo/concourse"):
    if _p not in sys.path:
        sys.path.insert(0, _p)

import numpy as np
import ml_dtypes

import concourse.bass as bass
import concourse.bacc as bacc
import concourse.mybir as mybir
import concourse.tile as tile
from concourse.bass import ds

AF = mybir.ActivationFunctionType
ALU = mybir.AluOpType
F32 = mybir.dt.float32
BF16 = mybir.dt.bfloat16
I16 = mybir.dt.int16

P = 128
B = 64          # total batch
NCORES = 8
BL = B // NCORES  # batch per core = 8
T = 512
E = 1024
H = 1024
G4 = 4 * H      # 4096
KC = E // P     # 8 contraction chunks
MC = G4 // P    # 32 output chunks of the gate dim
VOCAB = 32000

# Physical gate order on the 4H axis (chosen so the PE computes the gates in
# the order the elementwise chain consumes them: g, i, f, o).
# reference order in Wx/Wh rows: i, f, o, g  ->  physical: g, i, f, o
PHYS_GATES = (3, 0, 1, 2)  # orig block index for each physical block


def _build_program(n_blocks: int, tb: int, unroll: int):
    """Emit the bass/Tile program. tb = timesteps per block."""
    tok = tb * BL  # tokens (= matmul free dim) per block

    nc = bacc.Bacc(trn_type="TRN2", target_bir_lowering=False)

    emb_d = nc.dram_tensor("emb", [VOCAB, E], BF16, kind="ExternalInput")
    whT_d = nc.dram_tensor("whT", [KC, P, G4], BF16, kind="ExternalInput")
    wxT_d = nc.dram_tensor("wxT", [KC, P, G4], BF16, kind="ExternalInput")
    bxT_d = nc.dram_tensor("bxT", [P, MC], F32, kind="ExternalInput")
    idx_d = nc.dram_tensor("idx", [n_blocks, P, tok // 16], I16,
                           kind="ExternalInput")
    out_hT_d = nc.dram_tensor("out_hT", [n_blocks * tb * P, BL * KC], F32,
                              kind="ExternalOutput")
    out_c_d = nc.dram_tensor("out_c", [P, KC * BL], F32, kind="ExternalOutput")

    with tile.TileContext(nc) as tc:
        with (
            tc.tile_pool(name="const", bufs=1) as cpool,
            tc.tile_pool(name="state", bufs=1) as spool,
            tc.tile_pool(name="xt", bufs=2) as xtpool,
            tc.tile_pool(name="idx", bufs=2) as idxpool,
            tc.tile_pool(name="work", bufs=3) as wpool,
            tc.tile_pool(name="psg", bufs=2, space="PSUM") as psg_pool,
        ):
            # ---- resident tensors ----
            whT = cpool.tile([P, KC, G4], BF16)
            nc.sync.dma_start(whT[:], whT_d[:].rearrange("k p f -> p k f"))
            wxT = cpool.tile([P, KC, G4], BF16)
            nc.sync.dma_start(wxT[:], wxT_d[:].rearrange("k p f -> p k f"))
            bxT = cpool.tile([P, MC], F32)
            nc.sync.dma_start(bxT[:], bxT_d[:])

            xgT = spool.tile([P, MC, tok], BF16)

            # ping-pong state: step k writes slot k%2, reads slot (k+1)%2
            c_s = [spool.tile([P, KC, BL], F32, tag=f"c{s}", name=f"c{s}")
                   for s in range(2)]
            hT_s = [spool.tile([P, KC, BL], BF16, tag=f"h{s}", name=f"h{s}")
                    for s in range(2)]
            nc.vector.memset(c_s[1][:], 0.0)
            nc.vector.memset(hT_s[1][:], 0.0)

            def step(t_expr, k_parity, stage_slot):
                """One LSTM timestep. t_expr: dynamic step index within block
                (RuntimeValue or int), k_parity: global step parity.
                stage_slot: (P, KC, BL) f32 AP to write h into."""
                h_prev = hT_s[(k_parity + 1) % 2]
                c_prev = c_s[(k_parity + 1) % 2]
                c_new = c_s[k_parity]
                h_new_bf = hT_s[k_parity]

                # 4 per-gate PSUM tiles (each its own bank)
                ps = {}
                for gi, gname in enumerate("gifo"):
                    pt = psg_pool.tile([P, KC, BL], F32, tag=f"ps_{gname}")
                    ps[gname] = pt
                    for hc in range(KC):
                        mc = gi * KC + hc
                        for kc in range(KC):
                            nc.tensor.matmul(
                                pt[:, hc, :],
                                lhsT=whT[:, kc, mc * P:(mc + 1) * P],
                                rhs=h_prev[:, kc, :],
                                start=(kc == 0),
                                stop=(kc == KC - 1),
                            )

                tok0 = t_expr * BL

                def pre(gname, gi):
                    pr = wpool.tile([P, KC, BL], F32, tag=f"pre_{gname}")
                    nc.vector.tensor_tensor(
                        pr[:], ps[gname][:],
                        xgT[:, gi * KC:(gi + 1) * KC, ds(tok0, BL)],
                        ALU.add)
                    return pr

                # g gate: tanh
                g_pre = pre("g", 0)
                g_t = wpool.tile([P, KC, BL], F32, tag="g_t")
                nc.scalar.activation(g_t[:], g_pre[:], AF.Tanh)
                # i gate: sigmoid; m1 = i*g
                i_pre = pre("i", 1)
                i_s = wpool.tile([P, KC, BL], F32, tag="i_s")
                nc.scalar.activation(i_s[:], i_pre[:], AF.Sigmoid)
                m1 = wpool.tile([P, KC, BL], F32, tag="m1")
                nc.vector.tensor_tensor(m1[:], i_s[:], g_t[:], ALU.mult)
                # f gate: sigmoid; m2 = f*c_prev; c_new = m1 + m2
                f_pre = pre("f", 2)
                f_s = wpool.tile([P, KC, BL], F32, tag="f_s")
                nc.scalar.activation(f_s[:], f_pre[:], AF.Sigmoid)
                m2 = wpool.tile([P, KC, BL], F32, tag="m2")
                nc.vector.tensor_tensor(m2[:], f_s[:], c_prev[:], ALU.mult)
                nc.vector.tensor_tensor(c_new[:], m1[:], m2[:], ALU.add)
                # tanh(c)
                tc_t = wpool.tile([P, KC, BL], F32, tag="tc_t")
                nc.scalar.activation(tc_t[:], c_new[:], AF.Tanh)
                # o gate: sigmoid; h = o * tanh(c)
                o_pre = pre("o", 3)
                o_s = wpool.tile([P, KC, BL], F32, tag="o_s")
                nc.scalar.activation(o_s[:], o_pre[:], AF.Sigmoid)
                # bf16 h first: it feeds the next step's matmuls (critical path)
                nc.vector.tensor_tensor(h_new_bf[:], o_s[:], tc_t[:], ALU.mult)
                nc.vector.tensor_tensor(stage_slot, o_s[:], tc_t[:], ALU.mult)

            for kb in range(n_blocks):
                # ---- gather block embeddings, transposed, in bf16 ----
                idx_t = idxpool.tile([P, tok // 16], I16)
                nc.sync.dma_start(idx_t[:], idx_d[kb])
                xT = xtpool.tile([P, KC, tok], BF16)
                nc.gpsimd.dma_gather(
                    xT[:], emb_d[:], idx_t[:],
                    num_idxs=tok, num_idxs_reg=tok,
                    elem_size=E, transpose=True,
                )
                # ---- phase A: xgT = WxT.T @ xT + bx for the block ----
                # (reuses the 8 gate psum banks round-robin: same 1-bank slots)
                for mc in range(MC):
                    pxg = psg_pool.tile([P, tok], F32, tag=f"ps_{'gifo'[mc % 4]}",
                                        name="pxg")
                    for kc in range(KC):
                        nc.tensor.matmul(
                            pxg[:],
                            lhsT=wxT[:, kc, mc * P:(mc + 1) * P],
                            rhs=xT[:, kc, :],
                            start=(kc == 0),
                            stop=(kc == KC - 1),
                        )
                    nc.scalar.activation(
                        xgT[:, mc, :], pxg[:], AF.Identity,
                        bias=bxT[:, mc:mc + 1], scale=1.0)

                # ---- phase B: tb recurrent steps ----
                assert tb % unroll == 0 and unroll % 2 == 0

                def body(it_expr):
                    stage = wpool.tile([P, unroll, KC, BL], F32, tag="stage")
                    for u in range(unroll):
                        step(it_expr * unroll + u, u % 2, stage[:, u])
                    # one batched output DMA per body:
                    # dest rows (t, p) for t in [t0, t0+unroll)
                    row0 = (it_expr * unroll + kb * tb) * P
                    dst = out_hT_d[ds(row0, unroll * P), :]
                    dst = dst.rearrange("(u p) f -> p u f", p=P)
                    nc.sync.dma_start(
                        dst, stage[:].rearrange("p u a b -> p u (a b)"))

                if tb // unroll > 1:
                    with tc.For_i(0, tb // unroll, 1,
                                  hint_engines=(mybir.EngineType.PE,)) as it:
                        body(it)
                else:
                    body(0)

            # final c state lives in slot (last step parity) = 1 for even tb*n
            total_steps = n_blocks * tb
            final = (total_steps - 1) % 2
            nc.sync.dma_start(
                out_c_d[:], c_s[final][:].rearrange("p a b -> p (a b)"))

    nc.compile()
    return nc


_PROG_CACHE = {}


def _get_program(n_blocks, tb, unroll):
    key = (n_blocks, tb, unroll)
    if key not in _PROG_CACHE:
        _PROG_CACHE[key] = _build_program(n_blocks, tb, unroll)
    return _PROG_CACHE[key]


def prep_core_inputs(src_c, emb_bf, whT_np, wxT_np, bxT_np, n_blocks, tb):
    """Per-core host-side input prep. src_c: (BL, n_blocks*tb) int."""
    tok = tb * BL
    # token order within block: (t_local, b)
    idx = np.empty((n_blocks, P, tok // 16), dtype=np.int16)
    for kb in range(n_blocks):
        flat = src_c[:, kb * tb:(kb + 1) * tb].T.reshape(-1)  # (tb, BL) -> tok
        wrapped = flat.reshape(tok // 16, 16).T.astype(np.int16)  # [p, s]
        # the 8 GPSIMD Q7 cores each read their own 16-partition group
        idx[kb] = np.tile(wrapped, (P // 16, 1))
    return {
        "emb": emb_bf,
        "whT": whT_np,
        "wxT": wxT_np,
        "bxT": bxT_np,
        "idx": idx,
    }


def prep_weights(emb, Wx, bx, Wh):
    """Host-side weight permutation / transposition / cast (shared by cores)."""
    emb_bf = np.ascontiguousarray(emb.astype(ml_dtypes.bfloat16))

    def permute_rows(w):
        blocks = [w[g * H:(g + 1) * H] for g in range(4)]
        return np.concatenate([blocks[g] for g in PHYS_GATES], axis=0)

    Wh_p = permute_rows(Wh)          # (4H, H)
    Wx_p = permute_rows(Wx)          # (4H, E)
    bx_p = permute_rows(bx.reshape(4, H)).reshape(-1) \
        if False else np.concatenate([bx[g * H:(g + 1) * H] for g in PHYS_GATES])

    # whT[kc, p, j] = Wh_p[j, kc*P + p]
    whT_np = np.ascontiguousarray(
        Wh_p.T.reshape(KC, P, G4).astype(ml_dtypes.bfloat16))
    wxT_np = np.ascontiguousarray(
        Wx_p.T.reshape(KC, P, G4).astype(ml_dtypes.bfloat16))
    # bxT[p, mc] = bx_p[mc*P + p]
    bxT_np = np.ascontiguousarray(bx_p.reshape(MC, P).T.astype(np.float32))
    return emb_bf, whT_np, wxT_np, bxT_np


def postprocess(out_hT, out_c, n_blocks, tb):
    """out_hT: (n_blocks*tb*P, BL*KC) f32 -> (BL, T, H); out_c -> (BL, H)."""
    t_total = n_blocks * tb
    a = out_hT.reshape(t_total, P, KC, BL)          # [t, p, hc, b]
    outputs = np.ascontiguousarray(
        a.transpose(3, 0, 2, 1).reshape(BL, t_total, H))
    c = out_c.reshape(P, KC, BL).transpose(2, 1, 0).reshape(BL, H)
    return outputs, np.ascontiguousarray(c)


def _install_trace_hook():
    """The image's antenv package lacks axon_hooks; recreate it so
    run_bass_kernel_spmd(trace=True) can capture NTFF profiles."""
    import types
    if "antenv.axon_hooks" in sys.modules:
        return
    mod = types.ModuleType("antenv.axon_hooks")
    _h = [None]
    mod.set_axon_ntff_profile_hook = lambda h: _h.__setitem__(0, h)
    mod.get_axon_ntff_profile_hook = lambda: _h[0]
    sys.modules["antenv.axon_hooks"] = mod
    try:
        import antenv
        antenv.axon_hooks = mod
    except ImportError:
        pass
    try:
        from trn_agent_boot.trn_boot import _ntff_profile_via_ctypes
        mod.set_axon_ntff_profile_hook(
            _ntff_profile_via_ctypes("/opt/axon/libaxon_pjrt.so"))
    except Exception:
        pass


def kernel(src, emb, Wx, bx, Wh, *, n_blocks=8, tb=64, unroll=8, trace=False):
    from concourse.bass_utils import run_bass_kernel_spmd

    if trace:
        _install_trace_hook()

    src = np.asarray(src)
    emb = np.asarray(emb, dtype=np.float32)
    Wx = np.asarray(Wx, dtype=np.float32)
    bx = np.asarray(bx, dtype=np.float32)
    Wh = np.asarray(Wh, dtype=np.float32)

    t_total = n_blocks * tb
    assert src.shape == (B, T) and t_total <= T

    emb_bf, whT_np, wxT_np, bxT_np = prep_weights(emb, Wx, bx, Wh)

    nc = _get_program(n_blocks, tb, unroll)
    in_maps = []
    for c in range(NCORES):
        src_c = src[c * BL:(c + 1) * BL, :t_total]
        in_maps.append(prep_core_inputs(
            src_c, emb_bf, whT_np, wxT_np, bxT_np, n_blocks, tb))

    res = run_bass_kernel_spmd(
        nc, in_maps, core_ids=list(range(NCORES)), trace=trace)

    outs = np.empty((B, t_total, H), dtype=np.float32)
    c_T = np.empty((B, H), dtype=np.float32)
    for c in range(NCORES):
        o_c, c_c = postprocess(
            res.results[c]["out_hT"], res.results[c]["out_c"], n_blocks, tb)
        outs[c * BL:(c + 1) * BL] = o_c
        c_T[c * BL:(c + 1) * BL] = c_c
    h_T = np.ascontiguousarray(outs[:, -1, :])
    if trace:
        kernel.last_exec_time_ns = res.exec_time_ns
    return outs, h_T, c_T


# revision 22
# speedup vs baseline: 1.0783x; 1.0247x over previous
"""AttnLSTMEncoder Trainium2 kernel.

Computes, for src (B=64, T=512) int tokens:
    x  = emb[src]                       (B, T, E)
    xg = x @ Wx.T + bx                  (B, T, 4H)
    LSTM over T: gates = xg_t + h @ Wh.T ; i,f,o,g split; c = f*c + i*g;
                 h = o * tanh(c)
Returns (outputs (B,T,H) f32, h_T (B,H) f32, c_T (B,H) f32).

Strategy: data-parallel over batch across 8 NeuronCores (8 sequences per
core).  All on-chip tensors are kept transposed (H/4H dim on the 128 SBUF
partitions, batch on the free dim) so the scalar/vector engines run at full
lane utilization.  The recurrent matmul uses Wh tiles as the stationary
operand in bf16 (fast weight load), accumulating each gate into its own PSUM
bank so activations overlap the tensor engine.  The input-gate precompute
xg is done per 64-step block into SBUF (no DRAM round trip): embedding rows
are gathered transposed via dma_gather directly into (128, E/128, 512) bf16.
"""

import sys

for _p in ("/opt/trn_rl_repo", "/opt/trn_rl_repo/concourse"):
    if _p not in sys.path:
        sys.path.insert(0, _p)

import numpy as np
import ml_dtypes

import concourse.bass as bass
import concourse.bacc as bacc
import concourse.mybir as mybir
import concourse.tile as tile
from concourse.bass import ds

AF = mybir.ActivationFunctionType
ALU = mybir.AluOpType
F32 = mybir.dt.float32
BF16 = mybir.dt.bfloat16
I16 = mybir.dt.int16

P = 128
B = 64          # total batch
NCORES = 8
BL = B // NCORES  # batch per core = 8
T = 512
E = 1024
H = 1024
G4 = 4 * H      # 4096
KC = E // P     # 8 contraction chunks
MC = G4 // P    # 32 output chunks of the gate dim
VOCAB = 32000

# Physical gate order on the 4H axis (chosen so the PE computes the gates in
# the order the elementwise chain consumes them: g, i, f, o).
# reference order in Wx/Wh rows: i, f, o, g  ->  physical: g, i, f, o
PHYS_GATES = (3, 0, 1, 2)  # orig block index for each physical block


def _build_program(n_blocks: int, tb: int, unroll: int):
    """Emit the bass/Tile program. tb = timesteps per block."""
    tok = tb * BL  # tokens (= matmul free dim) per block

    nc = bacc.Bacc(trn_type="TRN2", target_bir_lowering=False)

    emb_d = nc.dram_tensor("emb", [VOCAB, E], BF16, kind="ExternalInput")
    whT_d = nc.dram_tensor("whT", [KC, P, G4], BF16, kind="ExternalInput")
    wxT_d = nc.dram_tensor("wxT", [KC, P, G4], BF16, kind="ExternalInput")
    bxT_d = nc.dram_tensor("bxT", [P, MC], F32, kind="ExternalInput")
    ident_d = nc.dram_tensor("ident", [P, P], BF16, kind="ExternalInput")
    idx_d = nc.dram_tensor("idx", [n_blocks, P, tok // 16], I16,
                           kind="ExternalInput")
    out_hT_d = nc.dram_tensor("out_hT", [n_blocks * tb * P, BL * KC], F32,
                              kind="ExternalOutput")
    out_c_d = nc.dram_tensor("out_c", [P, KC * BL], F32, kind="ExternalOutput")

    with tile.TileContext(nc) as tc:
        with (
            tc.tile_pool(name="const", bufs=1) as cpool,
            tc.tile_pool(name="state", bufs=1) as spool,
            tc.tile_pool(name="xt", bufs=2) as xtpool,
            tc.tile_pool(name="idx", bufs=2) as idxpool,
            tc.tile_pool(name="work", bufs=3) as wpool,
            tc.tile_pool(name="psg", bufs=2, space="PSUM") as psg_pool,
        ):
            # ---- resident tensors ----
            whT = cpool.tile([P, KC, G4], BF16)
            nc.sync.dma_start(whT[:], whT_d[:].rearrange("k p f -> p k f"))
            wxT = cpool.tile([P, KC, G4], BF16)
            nc.sync.dma_start(wxT[:], wxT_d[:].rearrange("k p f -> p k f"))
            bxT = cpool.tile([P, MC], F32)
            nc.sync.dma_start(bxT[:], bxT_d[:])
            ident = cpool.tile([P, P], BF16)
            nc.sync.dma_start(ident[:], ident_d[:])

            xgT = spool.tile([P, MC, tok], BF16)

            # ping-pong state: step k writes slot k%2, reads slot (k+1)%2
            c_s = [spool.tile([P, KC, BL], F32, tag=f"c{s}", name=f"c{s}")
                   for s in range(2)]
            hT_s = [spool.tile([P, KC, BL], BF16, tag=f"h{s}", name=f"h{s}")
                    for s in range(2)]
            nc.vector.memset(c_s[1][:], 0.0)
            nc.vector.memset(hT_s[1][:], 0.0)

            def step(t_expr, k_parity, stage_slot):
                """One LSTM timestep. t_expr: dynamic step index within block
                (RuntimeValue or int), k_parity: global step parity.
                stage_slot: (P, KC, BL) f32 AP to write h into."""
                h_prev = hT_s[(k_parity + 1) % 2]
                c_prev = c_s[(k_parity + 1) % 2]
                c_new = c_s[k_parity]
                h_new_bf = hT_s[k_parity]

                tok0 = nc.snap(t_expr * BL) if not isinstance(t_expr, int) \
                    else t_expr * BL

                # 4 per-gate PSUM tiles (each its own bank)
                ps = {}
                for gi, gname in enumerate("gifo"):
                    pt = psg_pool.tile([P, KC, BL], F32, tag=f"ps_{gname}")
                    ps[gname] = pt
                    last = gname == "o"
                    for hc in range(KC):
                        mc = gi * KC + hc
                        for kc in range(KC):
                            nc.tensor.matmul(
                                pt[:, hc, :],
                                lhsT=whT[:, kc, mc * P:(mc + 1) * P],
                                rhs=h_prev[:, kc, :],
                                start=(kc == 0),
                                stop=(kc == KC - 1) and not last,
                            )
                        if last:
                            # fold xg_o into PSUM so sigmoid can read PSUM
                            # directly - removes the DVE add from the
                            # critical tail between steps
                            nc.tensor.matmul(
                                pt[:, hc, :],
                                lhsT=ident[:],
                                rhs=xgT[:, mc, ds(tok0, BL)],
                                start=False, stop=True,
                            )

                def pre(gname, gi):
                    pr = wpool.tile([P, KC, BL], F32, tag=f"pre_{gname}")
                    nc.vector.tensor_tensor(
                        pr[:], ps[gname][:],
                        xgT[:, gi * KC:(gi + 1) * KC, ds(tok0, BL)],
                        ALU.add)
                    return pr

                # g gate: tanh
                g_pre = pre("g", 0)
                g_t = wpool.tile([P, KC, BL], F32, tag="g_t")
                nc.scalar.activation(g_t[:], g_pre[:], AF.Tanh)
                # i gate: sigmoid; m1 = i*g
                i_pre = pre("i", 1)
                i_s = wpool.tile([P, KC, BL], F32, tag="i_s")
                nc.scalar.activation(i_s[:], i_pre[:], AF.Sigmoid)
                m1 = wpool.tile([P, KC, BL], F32, tag="m1")
                nc.vector.tensor_tensor(m1[:], i_s[:], g_t[:], ALU.mult)
                # f gate: sigmoid; m2 = f*c_prev; c_new = m1 + m2
                f_pre = pre("f", 2)
                f_s = wpool.tile([P, KC, BL], F32, tag="f_s")
                nc.scalar.activation(f_s[:], f_pre[:], AF.Sigmoid)
                m2 = wpool.tile([P, KC, BL], F32, tag="m2")
                nc.vector.tensor_tensor(m2[:], f_s[:], c_prev[:], ALU.mult)
                nc.vector.tensor_tensor(c_new[:], m1[:], m2[:], ALU.add)
                # tanh(c)
                tc_t = wpool.tile([P, KC, BL], F32, tag="tc_t")
                nc.scalar.activation(tc_t[:], c_new[:], AF.Tanh)
                # o gate: sigmoid directly from PSUM (xg already folded in)
                o_s = wpool.tile([P, KC, BL], F32, tag="o_s")
                nc.scalar.activation(o_s[:], ps["o"][:], AF.Sigmoid)
                # bf16 h first: it feeds the next step's matmuls (critical path)
                nc.vector.tensor_tensor(h_new_bf[:], o_s[:], tc_t[:], ALU.mult)
                nc.vector.tensor_tensor(stage_slot, o_s[:], tc_t[:], ALU.mult)

            for kb in range(n_blocks):
                # ---- gather block embeddings, transposed, in bf16 ----
                idx_t = idxpool.tile([P, tok // 16], I16)
                nc.sync.dma_start(idx_t[:], idx_d[kb])
                xT = xtpool.tile([P, KC, tok], BF16)
                nc.gpsimd.dma_gather(
                    xT[:], emb_d[:], idx_t[:],
                    num_idxs=tok, num_idxs_reg=tok,
                    elem_size=E, transpose=True,
                )
                # ---- phase A: xgT = WxT.T @ xT + bx for the block ----
                # (reuses the 8 gate psum banks round-robin: same 1-bank slots)
                for mc in range(MC):
                    pxg = psg_pool.tile([P, tok], F32, tag=f"ps_{'gifo'[mc % 4]}",
                                        name="pxg")
                    for kc in range(KC):
                        nc.tensor.matmul(
                            pxg[:],
                            lhsT=wxT[:, kc, mc * P:(mc + 1) * P],
                            rhs=xT[:, kc, :],
                            start=(kc == 0),
                            stop=(kc == KC - 1),
                        )
                    nc.scalar.activation(
                        xgT[:, mc, :], pxg[:], AF.Identity,
                        bias=bxT[:, mc:mc + 1], scale=1.0)

                # ---- phase B: tb recurrent steps ----
                assert tb % unroll == 0 and unroll % 2 == 0

                def body(it_expr):
                    stage = wpool.tile([P, unroll, KC, BL], F32, tag="stage")
                    for u in range(unroll):
                        step(it_expr * unroll + u, u % 2, stage[:, u])
                    # one batched output DMA per body:
                    # dest rows (t, p) for t in [t0, t0+unroll)
                    row0 = (it_expr * unroll + kb * tb) * P
                    dst = out_hT_d[ds(row0, unroll * P), :]
                    dst = dst.rearrange("(u p) f -> p u f", p=P)
                    nc.sync.dma_start(
                        dst, stage[:].rearrange("p u a b -> p u (a b)"))

                if tb // unroll > 1:
                    with tc.For_i(0, tb // unroll, 1,
                                  hint_engines=(mybir.EngineType.PE,)) as it:
                        body(it)
                else:
                    body(0)

            # final c state lives in slot (last step parity) = 1 for even tb*n
            total_steps = n_blocks * tb
            final = (total_steps - 1) % 2
            nc.sync.dma_start(
                out_c_d[:], c_s[final][:].rearrange("p a b -> p (a b)"))

    nc.compile()
    return nc


_PROG_CACHE = {}


def _get_program(n_blocks, tb, unroll):
    key = (n_blocks, tb, unroll)
    if key not in _PROG_CACHE:
        _PROG_CACHE[key] = _build_program(n_blocks, tb, unroll)
    return _PROG_CACHE[key]


def prep_core_inputs(src_c, emb_bf, whT_np, wxT_np, bxT_np, n_blocks, tb):
    """Per-core host-side input prep. src_c: (BL, n_blocks*tb) int."""
    tok = tb * BL
    # token order within block: (t_local, b)
    idx = np.empty((n_blocks, P, tok // 16), dtype=np.int16)
    for kb in range(n_blocks):
        flat = src_c[:, kb * tb:(kb + 1) * tb].T.reshape(-1)  # (tb, BL) -> tok
        wrapped = flat.reshape(tok // 16, 16).T.astype(np.int16)  # [p, s]
        # the 8 GPSIMD Q7 cores each read their own 16-partition group
        idx[kb] = np.tile(wrapped, (P // 16, 1))
    return {
        "emb": emb_bf,
        "whT": whT_np,
        "wxT": wxT_np,
        "bxT": bxT_np,
        "idx": idx,
        "ident": np.eye(P, dtype=ml_dtypes.bfloat16),
    }


def prep_weights(emb, Wx, bx, Wh):
    """Host-side weight permutation / transposition / cast (shared by cores)."""
    emb_bf = np.ascontiguousarray(emb.astype(ml_dtypes.bfloat16))

    def permute_rows(w):
        blocks = [w[g * H:(g + 1) * H] for g in range(4)]
        return np.concatenate([blocks[g] for g in PHYS_GATES], axis=0)

    Wh_p = permute_rows(Wh)          # (4H, H)
    Wx_p = permute_rows(Wx)          # (4H, E)
    bx_p = permute_rows(bx.reshape(4, H)).reshape(-1) \
        if False else np.concatenate([bx[g * H:(g + 1) * H] for g in PHYS_GATES])

    # whT[kc, p, j] = Wh_p[j, kc*P + p]
    whT_np = np.ascontiguousarray(
        Wh_p.T.reshape(KC, P, G4).astype(ml_dtypes.bfloat16))
    wxT_np = np.ascontiguousarray(
        Wx_p.T.reshape(KC, P, G4).astype(ml_dtypes.bfloat16))
    # bxT[p, mc] = bx_p[mc*P + p]
    bxT_np = np.ascontiguousarray(bx_p.reshape(MC, P).T.astype(np.float32))
    return emb_bf, whT_np, wxT_np, bxT_np


def postprocess(out_hT, out_c, n_blocks, tb):
    """out_hT: (n_blocks*tb*P, BL*KC) f32 -> (BL, T, H); out_c -> (BL, H)."""
    t_total = n_blocks * tb
    a = out_hT.reshape(t_total, P, KC, BL)          # [t, p, hc, b]
    outputs = np.ascontiguousarray(
        a.transpose(3, 0, 2, 1).reshape(BL, t_total, H))
    c = out_c.reshape(P, KC, BL).transpose(2, 1, 0).reshape(BL, H)
    return outputs, np.ascontiguousarray(c)


def _install_trace_hook():
    """The image's antenv package lacks axon_hooks; recreate it so
    run_bass_kernel_spmd(trace=True) can capture NTFF profiles."""
    import types
    if "antenv.axon_hooks" in sys.modules:
        return
    mod = types.ModuleType("antenv.axon_hooks")
    _h = [None]
    mod.set_axon_ntff_profile_hook = lambda h: _h.__setitem__(0, h)
    mod.get_axon_ntff_profile_hook = lambda: _h[0]
    sys.modules["antenv.axon_hooks"] = mod
    try:
        import antenv
        antenv.axon_hooks = mod
    except ImportError:
        pass
    try:
        from trn_agent_boot.trn_boot import _ntff_profile_via_ctypes
        mod.set_axon_ntff_profile_hook(
            _ntff_profile_via_ctypes("/opt/axon/libaxon_pjrt.so"))
    except Exception:
        pass


def kernel(src, emb, Wx, bx, Wh, *, n_blocks=8, tb=64, unroll=8, trace=False):
    from concourse.bass_utils import run_bass_kernel_spmd

    if trace:
        _install_trace_hook()

    src = np.asarray(src)
    emb = np.asarray(emb, dtype=np.float32)
    Wx = np.asarray(Wx, dtype=np.float32)
    bx = np.asarray(bx, dtype=np.float32)
    Wh = np.asarray(Wh, dtype=np.float32)

    t_total = n_blocks * tb
    assert src.shape == (B, T) and t_total <= T

    emb_bf, whT_np, wxT_np, bxT_np = prep_weights(emb, Wx, bx, Wh)

    nc = _get_program(n_blocks, tb, unroll)
    in_maps = []
    for c in range(NCORES):
        src_c = src[c * BL:(c + 1) * BL, :t_total]
        in_maps.append(prep_core_inputs(
            src_c, emb_bf, whT_np, wxT_np, bxT_np, n_blocks, tb))

    res = run_bass_kernel_spmd(
        nc, in_maps, core_ids=list(range(NCORES)), trace=trace)

    outs = np.empty((B, t_total, H), dtype=np.float32)
    c_T = np.empty((B, H), dtype=np.float32)
    for c in range(NCORES):
        o_c, c_c = postprocess(
            res.results[c]["out_hT"], res.results[c]["out_c"], n_blocks, tb)
        outs[c * BL:(c + 1) * BL] = o_c
        c_T[c * BL:(c + 1) * BL] = c_c
    h_T = np.ascontiguousarray(outs[:, -1, :])
    if trace:
        kernel.last_exec_time_ns = res.exec_time_ns
    return outs, h_T, c_T


# revision 29
# speedup vs baseline: 1.0982x; 1.0184x over previous
"""AttnLSTMEncoder Trainium2 kernel.

Computes, for src (B=64, T=512) int tokens:
    x  = emb[src]                       (B, T, E)
    xg = x @ Wx.T + bx                  (B, T, 4H)
    LSTM over T: gates = xg_t + h @ Wh.T ; i,f,o,g split; c = f*c + i*g;
                 h = o * tanh(c)
Returns (outputs (B,T,H) f32, h_T (B,H) f32, c_T (B,H) f32).

Strategy: data-parallel over batch across 8 NeuronCores (8 sequences per
core).  All on-chip tensors are kept transposed (H/4H dim on the 128 SBUF
partitions, batch on the free dim) so the scalar/vector engines run at full
lane utilization.  The recurrent matmul uses Wh tiles as the stationary
operand in bf16 (fast weight load), accumulating each gate into its own PSUM
bank so activations overlap the tensor engine.  The input-gate precompute
xg is done per 64-step block into SBUF (no DRAM round trip): embedding rows
are gathered transposed via dma_gather directly into (128, E/128, 512) bf16.
"""

import sys

for _p in ("/opt/trn_rl_repo", "/opt/trn_rl_repo/concourse"):
    if _p not in sys.path:
        sys.path.insert(0, _p)

import numpy as np
import ml_dtypes

import concourse.bass as bass
import concourse.bacc as bacc
import concourse.mybir as mybir
import concourse.tile as tile
from concourse.bass import ds

AF = mybir.ActivationFunctionType
ALU = mybir.AluOpType
F32 = mybir.dt.float32
BF16 = mybir.dt.bfloat16
I16 = mybir.dt.int16

P = 128
B = 64          # total batch
NCORES = 8
BL = B // NCORES  # batch per core = 8
T = 512
E = 1024
H = 1024
G4 = 4 * H      # 4096
KC = E // P     # 8 contraction chunks
MC = G4 // P    # 32 output chunks of the gate dim
VOCAB = 32000

# Physical gate order on the 4H axis (chosen so the PE computes the gates in
# the order the elementwise chain consumes them: g, i, f, o).
# reference order in Wx/Wh rows: i, f, o, g  ->  physical: g, i, f, o
PHYS_GATES = (3, 0, 1, 2)  # orig block index for each physical block


def _build_program(n_blocks: int, tb: int, unroll: int):
    """Emit the bass/Tile program. tb = timesteps per block."""
    tok = tb * BL  # tokens (= matmul free dim) per block

    nc = bacc.Bacc(trn_type="TRN2", target_bir_lowering=False)

    emb_d = nc.dram_tensor("emb", [VOCAB, E], BF16, kind="ExternalInput")
    whT_d = nc.dram_tensor("whT", [KC, P, G4], BF16, kind="ExternalInput")
    wxT_d = nc.dram_tensor("wxT", [KC, P, G4], BF16, kind="ExternalInput")
    bxT_d = nc.dram_tensor("bxT", [P, MC], F32, kind="ExternalInput")
    ident_d = nc.dram_tensor("ident", [P, P], BF16, kind="ExternalInput")
    idx_d = nc.dram_tensor("idx", [n_blocks, P, tok // 16], I16,
                           kind="ExternalInput")
    out_hT_d = nc.dram_tensor("out_hT", [n_blocks * tb * P, BL * KC], F32,
                              kind="ExternalOutput")
    out_c_d = nc.dram_tensor("out_c", [P, KC * BL], F32, kind="ExternalOutput")

    with tile.TileContext(nc) as tc:
        with (
            tc.tile_pool(name="const", bufs=1) as cpool,
            tc.tile_pool(name="state", bufs=1) as spool,
            tc.tile_pool(name="xt", bufs=2) as xtpool,
            tc.tile_pool(name="idx", bufs=2) as idxpool,
            tc.tile_pool(name="work", bufs=3) as wpool,
            tc.tile_pool(name="psg", bufs=2, space="PSUM") as psg_pool,
        ):
            # ---- resident tensors ----
            whT = cpool.tile([P, KC, G4], BF16)
            nc.sync.dma_start(whT[:], whT_d[:].rearrange("k p f -> p k f"))
            wxT = cpool.tile([P, KC, G4], BF16)
            nc.sync.dma_start(wxT[:], wxT_d[:].rearrange("k p f -> p k f"))
            bxT = cpool.tile([P, MC], F32)
            nc.sync.dma_start(bxT[:], bxT_d[:])
            ident = cpool.tile([P, P], BF16)
            nc.sync.dma_start(ident[:], ident_d[:])

            xgT = spool.tile([P, MC, tok], BF16)

            # ping-pong state: step k writes slot k%2, reads slot (k+1)%2
            c_s = [spool.tile([P, KC, BL], F32, tag=f"c{s}", name=f"c{s}")
                   for s in range(2)]
            hT_s = [spool.tile([P, KC, BL], BF16, tag=f"h{s}", name=f"h{s}")
                    for s in range(2)]
            nc.vector.memset(c_s[1][:], 0.0)
            nc.vector.memset(hT_s[1][:], 0.0)

            def step(t_expr, k_parity, stage_slot):
                """One LSTM timestep. t_expr: dynamic step index within block
                (RuntimeValue or int), k_parity: global step parity.
                stage_slot: (P, KC, BL) f32 AP to write h into."""
                h_prev = hT_s[(k_parity + 1) % 2]
                c_prev = c_s[(k_parity + 1) % 2]
                c_new = c_s[k_parity]
                h_new_bf = hT_s[k_parity]

                tok0 = nc.snap(t_expr * BL) if not isinstance(t_expr, int) \
                    else t_expr * BL

                # 4 per-gate PSUM tiles (each its own bank).  Each gate's xg
                # contribution is folded into PSUM with an identity matmul so
                # the activations read PSUM directly (no DVE add on the
                # critical path).
                ps = {}
                for gi, gname in enumerate("gifo"):
                    pt = psg_pool.tile([P, KC, BL], F32, tag=f"ps_{gname}")
                    ps[gname] = pt
                    # xg for the whole gate enters PSUM first (one identity
                    # matmul, start=True) - it has no dependency on h, so the
                    # PE can issue it while waiting for the previous step.
                    nc.tensor.matmul(
                        pt[:, :, :],
                        lhsT=ident[:],
                        rhs=xgT[:, gi * KC:(gi + 1) * KC, ds(tok0, BL)],
                        start=True, stop=True,
                    )
                    for hc in range(KC):
                        mc = gi * KC + hc
                        for kc in range(KC):
                            nc.tensor.matmul(
                                pt[:, hc, :],
                                lhsT=whT[:, kc, mc * P:(mc + 1) * P],
                                rhs=h_prev[:, kc, :],
                                start=False,
                                stop=False,
                                skip_group_check=True,
                            )

                # g gate: tanh
                g_t = wpool.tile([P, KC, BL], F32, tag="g_t")
                nc.scalar.activation(g_t[:], ps["g"][:], AF.Tanh)
                # i gate: sigmoid; m1 = i*g
                i_s = wpool.tile([P, KC, BL], F32, tag="i_s")
                nc.scalar.activation(i_s[:], ps["i"][:], AF.Sigmoid)
                m1 = wpool.tile([P, KC, BL], F32, tag="m1")
                nc.vector.tensor_tensor(m1[:], i_s[:], g_t[:], ALU.mult)
                # f gate: sigmoid; m2 = f*c_prev; c_new = m1 + m2
                f_s = wpool.tile([P, KC, BL], F32, tag="f_s")
                nc.scalar.activation(f_s[:], ps["f"][:], AF.Sigmoid)
                m2 = wpool.tile([P, KC, BL], F32, tag="m2")
                nc.vector.tensor_tensor(m2[:], f_s[:], c_prev[:], ALU.mult)
                nc.vector.tensor_tensor(c_new[:], m1[:], m2[:], ALU.add)
                # tanh(c)
                tc_t = wpool.tile([P, KC, BL], F32, tag="tc_t")
                nc.scalar.activation(tc_t[:], c_new[:], AF.Tanh)
                # o gate: sigmoid directly from PSUM (xg already folded in)
                o_s = wpool.tile([P, KC, BL], F32, tag="o_s")
                nc.scalar.activation(o_s[:], ps["o"][:], AF.Sigmoid)
                # bf16 h first: it feeds the next step's matmuls (critical path)
                nc.vector.tensor_tensor(h_new_bf[:], o_s[:], tc_t[:], ALU.mult)
                nc.vector.tensor_tensor(stage_slot, o_s[:], tc_t[:], ALU.mult)

            for kb in range(n_blocks):
                # ---- gather block embeddings, transposed, in bf16 ----
                idx_t = idxpool.tile([P, tok // 16], I16)
                nc.sync.dma_start(idx_t[:], idx_d[kb])
                xT = xtpool.tile([P, KC, tok], BF16)
                nc.gpsimd.dma_gather(
                    xT[:], emb_d[:], idx_t[:],
                    num_idxs=tok, num_idxs_reg=tok,
                    elem_size=E, transpose=True,
                )
                # ---- phase A: xgT = WxT.T @ xT + bx for the block ----
                # (reuses the 8 gate psum banks round-robin: same 1-bank slots)
                for mc in range(MC):
                    pxg = psg_pool.tile([P, tok], F32, tag=f"ps_{'gifo'[mc % 4]}",
                                        name="pxg")
                    for kc in range(KC):
                        nc.tensor.matmul(
                            pxg[:],
                            lhsT=wxT[:, kc, mc * P:(mc + 1) * P],
                            rhs=xT[:, kc, :],
                            start=(kc == 0),
                            stop=(kc == KC - 1),
                        )
                    nc.scalar.activation(
                        xgT[:, mc, :], pxg[:], AF.Identity,
                        bias=bxT[:, mc:mc + 1], scale=1.0)

                # ---- phase B: tb recurrent steps ----
                assert tb % unroll == 0 and unroll % 2 == 0

                def body(it_expr):
                    stage = wpool.tile([P, unroll, KC, BL], F32, tag="stage")
                    for u in range(unroll):
                        step(it_expr * unroll + u, u % 2, stage[:, u])
                    # one batched output DMA per body:
                    # dest rows (t, p) for t in [t0, t0+unroll)
                    row0 = (it_expr * unroll + kb * tb) * P
                    dst = out_hT_d[ds(row0, unroll * P), :]
                    dst = dst.rearrange("(u p) f -> p u f", p=P)
                    nc.sync.dma_start(
                        dst, stage[:].rearrange("p u a b -> p u (a b)"))

                if tb // unroll > 1:
                    with tc.For_i(0, tb // unroll, 1,
                                  hint_engines=(mybir.EngineType.PE,),
                                  staggered_reset=True) as it:
                        body(it)
                else:
                    body(0)

            # final c state lives in slot (last step parity) = 1 for even tb*n
            total_steps = n_blocks * tb
            final = (total_steps - 1) % 2
            nc.sync.dma_start(
                out_c_d[:], c_s[final][:].rearrange("p a b -> p (a b)"))

    nc.compile()
    return nc


_PROG_CACHE = {}


def _get_program(n_blocks, tb, unroll):
    key = (n_blocks, tb, unroll)
    if key not in _PROG_CACHE:
        _PROG_CACHE[key] = _build_program(n_blocks, tb, unroll)
    return _PROG_CACHE[key]


def prep_core_inputs(src_c, emb_bf, whT_np, wxT_np, bxT_np, n_blocks, tb):
    """Per-core host-side input prep. src_c: (BL, n_blocks*tb) int."""
    tok = tb * BL
    # token order within block: (t_local, b)
    idx = np.empty((n_blocks, P, tok // 16), dtype=np.int16)
    for kb in range(n_blocks):
        flat = src_c[:, kb * tb:(kb + 1) * tb].T.reshape(-1)  # (tb, BL) -> tok
        wrapped = flat.reshape(tok // 16, 16).T.astype(np.int16)  # [p, s]
        # the 8 GPSIMD Q7 cores each read their own 16-partition group
        idx[kb] = np.tile(wrapped, (P // 16, 1))
    return {
        "emb": emb_bf,
        "whT": whT_np,
        "wxT": wxT_np,
        "bxT": bxT_np,
        "idx": idx,
        "ident": np.eye(P, dtype=ml_dtypes.bfloat16),
    }


def prep_weights(emb, Wx, bx, Wh):
    """Host-side weight permutation / transposition / cast (shared by cores)."""
    emb_bf = np.ascontiguousarray(emb.astype(ml_dtypes.bfloat16))

    def permute_rows(w):
        blocks = [w[g * H:(g + 1) * H] for g in range(4)]
        return np.concatenate([blocks[g] for g in PHYS_GATES], axis=0)

    Wh_p = permute_rows(Wh)          # (4H, H)
    Wx_p = permute_rows(Wx)          # (4H, E)
    bx_p = permute_rows(bx.reshape(4, H)).reshape(-1) \
        if False else np.concatenate([bx[g * H:(g + 1) * H] for g in PHYS_GATES])

    # whT[kc, p, j] = Wh_p[j, kc*P + p]
    whT_np = np.ascontiguousarray(
        Wh_p.T.reshape(KC, P, G4).astype(ml_dtypes.bfloat16))
    wxT_np = np.ascontiguousarray(
        Wx_p.T.reshape(KC, P, G4).astype(ml_dtypes.bfloat16))
    # bxT[p, mc] = bx_p[mc*P + p]
    bxT_np = np.ascontiguousarray(bx_p.reshape(MC, P).T.astype(np.float32))
    return emb_bf, whT_np, wxT_np, bxT_np


def postprocess(out_hT, out_c, n_blocks, tb):
    """out_hT: (n_blocks*tb*P, BL*KC) f32 -> (BL, T, H); out_c -> (BL, H)."""
    t_total = n_blocks * tb
    a = out_hT.reshape(t_total, P, KC, BL)          # [t, p, hc, b]
    outputs = np.ascontiguousarray(
        a.transpose(3, 0, 2, 1).reshape(BL, t_total, H))
    c = out_c.reshape(P, KC, BL).transpose(2, 1, 0).reshape(BL, H)
    return outputs, np.ascontiguousarray(c)


def _install_trace_hook():
    """The image's antenv package lacks axon_hooks; recreate it so
    run_bass_kernel_spmd(trace=True) can capture NTFF profiles."""
    import types
    if "antenv.axon_hooks" in sys.modules:
        return
    mod = types.ModuleType("antenv.axon_hooks")
    _h = [None]
    mod.set_axon_ntff_profile_hook = lambda h: _h.__setitem__(0, h)
    mod.get_axon_ntff_profile_hook = lambda: _h[0]
    sys.modules["antenv.axon_hooks"] = mod
    try:
        import antenv
        antenv.axon_hooks = mod
    except ImportError:
        pass
    try:
        from trn_agent_boot.trn_boot import _ntff_profile_via_ctypes
        mod.set_axon_ntff_profile_hook(
            _ntff_profile_via_ctypes("/opt/axon/libaxon_pjrt.so"))
    except Exception:
        pass


def kernel(src, emb, Wx, bx, Wh, *, n_blocks=8, tb=64, unroll=8, trace=False):
    from concourse.bass_utils import run_bass_kernel_spmd

    if trace:
        _install_trace_hook()

    src = np.asarray(src)
    emb = np.asarray(emb, dtype=np.float32)
    Wx = np.asarray(Wx, dtype=np.float32)
    bx = np.asarray(bx, dtype=np.float32)
    Wh = np.asarray(Wh, dtype=np.float32)

    t_total = n_blocks * tb
    assert src.shape == (B, T) and t_total <= T

    emb_bf, whT_np, wxT_np, bxT_np = prep_weights(emb, Wx, bx, Wh)

    nc = _get_program(n_blocks, tb, unroll)
    in_maps = []
    for c in range(NCORES):
        src_c = src[c * BL:(c + 1) * BL, :t_total]
        in_maps.append(prep_core_inputs(
            src_c, emb_bf, whT_np, wxT_np, bxT_np, n_blocks, tb))

    res = run_bass_kernel_spmd(
        nc, in_maps, core_ids=list(range(NCORES)), trace=trace)

    outs = np.empty((B, t_total, H), dtype=np.float32)
    c_T = np.empty((B, H), dtype=np.float32)
    for c in range(NCORES):
        o_c, c_c = postprocess(
            res.results[c]["out_hT"], res.results[c]["out_c"], n_blocks, tb)
        outs[c * BL:(c + 1) * BL] = o_c
        c_T[c * BL:(c + 1) * BL] = c_c
    h_T = np.ascontiguousarray(outs[:, -1, :])
    if trace:
        kernel.last_exec_time_ns = res.exec_time_ns
    return outs, h_T, c_T


# revision 31
# speedup vs baseline: 1.1503x; 1.0474x over previous
"""AttnLSTMEncoder Trainium2 kernel.

Computes, for src (B=64, T=512) int tokens:
    x  = emb[src]                       (B, T, E)
    xg = x @ Wx.T + bx                  (B, T, 4H)
    LSTM over T: gates = xg_t + h @ Wh.T ; i,f,o,g split; c = f*c + i*g;
                 h = o * tanh(c)
Returns (outputs (B,T,H) f32, h_T (B,H) f32, c_T (B,H) f32).

Strategy: data-parallel over batch across 8 NeuronCores (8 sequences per
core).  All on-chip tensors are kept transposed (H/4H dim on the 128 SBUF
partitions, batch on the free dim) so the scalar/vector engines run at full
lane utilization.  The recurrent matmul uses Wh tiles as the stationary
operand in bf16 (fast weight load), accumulating each gate into its own PSUM
bank so activations overlap the tensor engine.  The input-gate precompute
xg is done per 64-step block into SBUF (no DRAM round trip): embedding rows
are gathered transposed via dma_gather directly into (128, E/128, 512) bf16.
"""

import sys

for _p in ("/opt/trn_rl_repo", "/opt/trn_rl_repo/concourse"):
    if _p not in sys.path:
        sys.path.insert(0, _p)

import numpy as np
import ml_dtypes

import concourse.bass as bass
import concourse.bacc as bacc
import concourse.mybir as mybir
import concourse.tile as tile
from concourse.bass import ds
from concourse.tile_rust import add_dep_helper

AF = mybir.ActivationFunctionType
ALU = mybir.AluOpType
F32 = mybir.dt.float32
BF16 = mybir.dt.bfloat16
I16 = mybir.dt.int16

P = 128
B = 64          # total batch
NCORES = 8
BL = B // NCORES  # batch per core = 8
T = 512
E = 1024
H = 1024
G4 = 4 * H      # 4096
KC = E // P     # 8 contraction chunks
MC = G4 // P    # 32 output chunks of the gate dim
VOCAB = 32000

# Physical gate order on the 4H axis (chosen so the PE computes the gates in
# the order the elementwise chain consumes them: g, i, f, o).
# reference order in Wx/Wh rows: i, f, o, g  ->  physical: g, i, f, o
PHYS_GATES = (3, 0, 1, 2)  # orig block index for each physical block


def _build_program(n_blocks: int, tb: int, unroll: int):
    """Emit the bass/Tile program. tb = timesteps per block."""
    tok = tb * BL  # tokens (= matmul free dim) per block

    nc = bacc.Bacc(trn_type="TRN2", target_bir_lowering=False)

    emb_d = nc.dram_tensor("emb", [VOCAB, E], BF16, kind="ExternalInput")
    whT_d = nc.dram_tensor("whT", [KC, P, G4], BF16, kind="ExternalInput")
    wxT_d = nc.dram_tensor("wxT", [KC, P, G4], BF16, kind="ExternalInput")
    bxT_d = nc.dram_tensor("bxT", [P, MC], F32, kind="ExternalInput")
    ident_d = nc.dram_tensor("ident", [P, P], BF16, kind="ExternalInput")
    idx_d = nc.dram_tensor("idx", [n_blocks, P, tok // 16], I16,
                           kind="ExternalInput")
    out_hT_d = nc.dram_tensor("out_hT", [n_blocks * tb * P, BL * KC], F32,
                              kind="ExternalOutput")
    out_c_d = nc.dram_tensor("out_c", [P, KC * BL], F32, kind="ExternalOutput")

    with tile.TileContext(nc) as tc:
        with (
            tc.tile_pool(name="const", bufs=1) as cpool,
            tc.tile_pool(name="state", bufs=1) as spool,
            tc.tile_pool(name="xt", bufs=2) as xtpool,
            tc.tile_pool(name="idx", bufs=2) as idxpool,
            tc.tile_pool(name="work", bufs=3) as wpool,
            tc.tile_pool(name="psg", bufs=2, space="PSUM") as psg_pool,
        ):
            # ---- resident tensors ----
            whT = cpool.tile([P, KC, G4], BF16)
            nc.sync.dma_start(whT[:], whT_d[:].rearrange("k p f -> p k f"))
            wxT = cpool.tile([P, KC, G4], BF16)
            nc.sync.dma_start(wxT[:], wxT_d[:].rearrange("k p f -> p k f"))
            bxT = cpool.tile([P, MC], F32)
            nc.sync.dma_start(bxT[:], bxT_d[:])
            ident = cpool.tile([P, P], BF16)
            nc.sync.dma_start(ident[:], ident_d[:])

            xgT = spool.tile([P, MC, tok], BF16)

            # ping-pong state: step k writes slot k%2, reads slot (k+1)%2
            c_s = [spool.tile([P, KC, BL], F32, tag=f"c{s}", name=f"c{s}")
                   for s in range(2)]
            hT_s = [spool.tile([P, KC, BL], BF16, tag=f"h{s}", name=f"h{s}")
                    for s in range(2)]
            nc.vector.memset(c_s[1][:], 0.0)
            nc.vector.memset(hT_s[1][:], 0.0)

            def step(t_expr, k_parity, stage_slot):
                """One LSTM timestep. t_expr: dynamic step index within block
                (RuntimeValue or int), k_parity: global step parity.
                stage_slot: (P, KC, BL) f32 AP to write h into."""
                h_prev = hT_s[(k_parity + 1) % 2]
                c_prev = c_s[(k_parity + 1) % 2]
                c_new = c_s[k_parity]
                h_new_bf = hT_s[k_parity]

                tok0 = nc.snap(t_expr * BL) if not isinstance(t_expr, int) \
                    else t_expr * BL

                # 4 per-gate PSUM tiles (each its own bank).  Each gate's xg
                # contribution is folded into PSUM with an identity matmul so
                # the activations read PSUM directly (no DVE add on the
                # critical path).
                ps = {}
                for gi, gname in enumerate("gifo"):
                    pt = psg_pool.tile([P, KC, BL], F32, tag=f"ps_{gname}")
                    ps[gname] = pt
                    # xg for the whole gate enters PSUM first (one identity
                    # matmul, start=True) - it has no dependency on h, so the
                    # PE can issue it while waiting for the previous step.
                    nc.tensor.matmul(
                        pt[:, :, :],
                        lhsT=ident[:],
                        rhs=xgT[:, gi * KC:(gi + 1) * KC, ds(tok0, BL)],
                        start=True, stop=True,
                    )
                    for hc in range(KC):
                        mc = gi * KC + hc
                        for kc in range(KC):
                            nc.tensor.matmul(
                                pt[:, hc, :],
                                lhsT=whT[:, kc, mc * P:(mc + 1) * P],
                                rhs=h_prev[:, kc, :],
                                start=False,
                                stop=False,
                                skip_group_check=True,
                            )

                # g gate: tanh
                g_t = wpool.tile([P, KC, BL], F32, tag="g_t")
                nc.scalar.activation(g_t[:], ps["g"][:], AF.Tanh)
                # i gate: sigmoid; m1 = i*g
                i_s = wpool.tile([P, KC, BL], F32, tag="i_s")
                nc.scalar.activation(i_s[:], ps["i"][:], AF.Sigmoid)
                m1 = wpool.tile([P, KC, BL], F32, tag="m1")
                nc.vector.tensor_tensor(m1[:], i_s[:], g_t[:], ALU.mult)
                # f gate: sigmoid; m2 = f*c_prev; c_new = m1 + m2
                f_s = wpool.tile([P, KC, BL], F32, tag="f_s")
                nc.scalar.activation(f_s[:], ps["f"][:], AF.Sigmoid)
                m2 = wpool.tile([P, KC, BL], F32, tag="m2")
                nc.vector.tensor_tensor(m2[:], f_s[:], c_prev[:], ALU.mult)
                nc.vector.tensor_tensor(c_new[:], m1[:], m2[:], ALU.add)
                # tanh(c)
                tc_t = wpool.tile([P, KC, BL], F32, tag="tc_t")
                tanh_c_i = nc.scalar.activation(tc_t[:], c_new[:], AF.Tanh)
                # o gate: sigmoid directly from PSUM (xg already folded in)
                o_s = wpool.tile([P, KC, BL], F32, tag="o_s")
                sig_o_i = nc.scalar.activation(o_s[:], ps["o"][:], AF.Sigmoid)
                # keep ACT queue order tanh_c -> sig_o: tanh_c is ready long
                # before o's matmuls finish; the scheduler's cost model
                # otherwise reorders them and delays the critical tail
                add_dep_helper(sig_o_i.ins, tanh_c_i.ins, sync=False,
                               reason="tail order: tanh_c before sig_o")
                # bf16 h first: it feeds the next step's matmuls (critical path)
                nc.vector.tensor_tensor(h_new_bf[:], o_s[:], tc_t[:], ALU.mult)
                nc.vector.tensor_tensor(stage_slot, o_s[:], tc_t[:], ALU.mult)

            for kb in range(n_blocks):
                # ---- gather block embeddings, transposed, in bf16 ----
                idx_t = idxpool.tile([P, tok // 16], I16)
                nc.sync.dma_start(idx_t[:], idx_d[kb])
                xT = xtpool.tile([P, KC, tok], BF16)
                nc.gpsimd.dma_gather(
                    xT[:], emb_d[:], idx_t[:],
                    num_idxs=tok, num_idxs_reg=tok,
                    elem_size=E, transpose=True,
                )
                # ---- phase A: xgT = WxT.T @ xT + bx for the block ----
                # (reuses the 8 gate psum banks round-robin: same 1-bank slots)
                for mc in range(MC):
                    pxg = psg_pool.tile([P, tok], F32, tag=f"ps_{'gifo'[mc % 4]}",
                                        name="pxg")
                    for kc in range(KC):
                        nc.tensor.matmul(
                            pxg[:],
                            lhsT=wxT[:, kc, mc * P:(mc + 1) * P],
                            rhs=xT[:, kc, :],
                            start=(kc == 0),
                            stop=(kc == KC - 1),
                        )
                    nc.scalar.activation(
                        xgT[:, mc, :], pxg[:], AF.Identity,
                        bias=bxT[:, mc:mc + 1], scale=1.0)

                # ---- phase B: tb recurrent steps ----
                assert tb % unroll == 0 and unroll % 2 == 0

                def body(it_expr):
                    stage = wpool.tile([P, unroll, KC, BL], F32, tag="stage")
                    for u in range(unroll):
                        step(it_expr * unroll + u, u % 2, stage[:, u])
                    # one batched output DMA per body:
                    # dest rows (t, p) for t in [t0, t0+unroll)
                    row0 = (it_expr * unroll + kb * tb) * P
                    dst = out_hT_d[ds(row0, unroll * P), :]
                    dst = dst.rearrange("(u p) f -> p u f", p=P)
                    nc.sync.dma_start(
                        dst, stage[:].rearrange("p u a b -> p u (a b)"))

                if tb // unroll > 1:
                    with tc.For_i(0, tb // unroll, 1,
                                  hint_engines=(mybir.EngineType.PE,),
                                  staggered_reset=True) as it:
                        body(it)
                else:
                    body(0)

            # final c state lives in slot (last step parity) = 1 for even tb*n
            total_steps = n_blocks * tb
            final = (total_steps - 1) % 2
            nc.sync.dma_start(
                out_c_d[:], c_s[final][:].rearrange("p a b -> p (a b)"))

    nc.compile()
    return nc


_PROG_CACHE = {}


def _get_program(n_blocks, tb, unroll):
    key = (n_blocks, tb, unroll)
    if key not in _PROG_CACHE:
        _PROG_CACHE[key] = _build_program(n_blocks, tb, unroll)
    return _PROG_CACHE[key]


def prep_core_inputs(src_c, emb_bf, whT_np, wxT_np, bxT_np, n_blocks, tb):
    """Per-core host-side input prep. src_c: (BL, n_blocks*tb) int."""
    tok = tb * BL
    # token order within block: (t_local, b)
    idx = np.empty((n_blocks, P, tok // 16), dtype=np.int16)
    for kb in range(n_blocks):
        flat = src_c[:, kb * tb:(kb + 1) * tb].T.reshape(-1)  # (tb, BL) -> tok
        wrapped = flat.reshape(tok // 16, 16).T.astype(np.int16)  # [p, s]
        # the 8 GPSIMD Q7 cores each read their own 16-partition group
        idx[kb] = np.tile(wrapped, (P // 16, 1))
    return {
        "emb": emb_bf,
        "whT": whT_np,
        "wxT": wxT_np,
        "bxT": bxT_np,
        "idx": idx,
        "ident": np.eye(P, dtype=ml_dtypes.bfloat16),
    }


def prep_weights(emb, Wx, bx, Wh):
    """Host-side weight permutation / transposition / cast (shared by cores)."""
    emb_bf = np.ascontiguousarray(emb.astype(ml_dtypes.bfloat16))

    def permute_rows(w):
        blocks = [w[g * H:(g + 1) * H] for g in range(4)]
        return np.concatenate([blocks[g] for g in PHYS_GATES], axis=0)

    Wh_p = permute_rows(Wh)          # (4H, H)
    Wx_p = permute_rows(Wx)          # (4H, E)
    bx_p = permute_rows(bx.reshape(4, H)).reshape(-1) \
        if False else np.concatenate([bx[g * H:(g + 1) * H] for g in PHYS_GATES])

    # whT[kc, p, j] = Wh_p[j, kc*P + p]
    whT_np = np.ascontiguousarray(
        Wh_p.T.reshape(KC, P, G4).astype(ml_dtypes.bfloat16))
    wxT_np = np.ascontiguousarray(
        Wx_p.T.reshape(KC, P, G4).astype(ml_dtypes.bfloat16))
    # bxT[p, mc] = bx_p[mc*P + p]
    bxT_np = np.ascontiguousarray(bx_p.reshape(MC, P).T.astype(np.float32))
    return emb_bf, whT_np, wxT_np, bxT_np


def postprocess(out_hT, out_c, n_blocks, tb):
    """out_hT: (n_blocks*tb*P, BL*KC) f32 -> (BL, T, H); out_c -> (BL, H)."""
    t_total = n_blocks * tb
    a = out_hT.reshape(t_total, P, KC, BL)          # [t, p, hc, b]
    outputs = np.ascontiguousarray(
        a.transpose(3, 0, 2, 1).reshape(BL, t_total, H))
    c = out_c.reshape(P, KC, BL).transpose(2, 1, 0).reshape(BL, H)
    return outputs, np.ascontiguousarray(c)


def _install_trace_hook():
    """The image's antenv package lacks axon_hooks; recreate it so
    run_bass_kernel_spmd(trace=True) can capture NTFF profiles."""
    import types
    if "antenv.axon_hooks" in sys.modules:
        return
    mod = types.ModuleType("antenv.axon_hooks")
    _h = [None]
    mod.set_axon_ntff_profile_hook = lambda h: _h.__setitem__(0, h)
    mod.get_axon_ntff_profile_hook = lambda: _h[0]
    sys.modules["antenv.axon_hooks"] = mod
    try:
        import antenv
        antenv.axon_hooks = mod
    except ImportError:
        pass
    try:
        from trn_agent_boot.trn_boot import _ntff_profile_via_ctypes
        mod.set_axon_ntff_profile_hook(
            _ntff_profile_via_ctypes("/opt/axon/libaxon_pjrt.so"))
    except Exception:
        pass


def kernel(src, emb, Wx, bx, Wh, *, n_blocks=8, tb=64, unroll=8, trace=False):
    from concourse.bass_utils import run_bass_kernel_spmd

    if trace:
        _install_trace_hook()

    src = np.asarray(src)
    emb = np.asarray(emb, dtype=np.float32)
    Wx = np.asarray(Wx, dtype=np.float32)
    bx = np.asarray(bx, dtype=np.float32)
    Wh = np.asarray(Wh, dtype=np.float32)

    t_total = n_blocks * tb
    assert src.shape == (B, T) and t_total <= T

    emb_bf, whT_np, wxT_np, bxT_np = prep_weights(emb, Wx, bx, Wh)

    nc = _get_program(n_blocks, tb, unroll)
    in_maps = []
    for c in range(NCORES):
        src_c = src[c * BL:(c + 1) * BL, :t_total]
        in_maps.append(prep_core_inputs(
            src_c, emb_bf, whT_np, wxT_np, bxT_np, n_blocks, tb))

    res = run_bass_kernel_spmd(
        nc, in_maps, core_ids=list(range(NCORES)), trace=trace)

    outs = np.empty((B, t_total, H), dtype=np.float32)
    c_T = np.empty((B, H), dtype=np.float32)
    for c in range(NCORES):
        o_c, c_c = postprocess(
            res.results[c]["out_hT"], res.results[c]["out_c"], n_blocks, tb)
        outs[c * BL:(c + 1) * BL] = o_c
        c_T[c * BL:(c + 1) * BL] = c_c
    h_T = np.ascontiguousarray(outs[:, -1, :])
    if trace:
        kernel.last_exec_time_ns = res.exec_time_ns
    return outs, h_T, c_T
